# revision 48
# baseline (speedup 1.0000x reference)
"""BERT self-attention (no mask) on 8 TRN2 NeuronCores, head-parallel.

Full inputs in, full output out. Core c computes heads 2c and 2c+1, i.e.
output hidden columns [c*128, (c+1)*128). Matmul operands are float32r
(full-rate near-fp32 streaming; producers round on write). Attention is
computed in transposed layout (scores^T[k, q]) so the softmax
denominator comes out of the PV matmul for free via a ones-column
appended to V. Projection (per batch) and attention (previous batch)
are interleaved so TensorE fills the gaps of the ACT-bound exp stream.
"""

import numpy as np

try:
    import concourse.bass as bass
except ImportError:  # toolchain not on sys.path in the caller's environment
    import sys
    sys.path.insert(0, "/opt/trn_rl_repo")
    import concourse.bass as bass
import concourse.bacc as bacc
import concourse.mybir as mybir
import concourse.tile as tile
from concourse.bass_utils import run_bass_kernel_spmd
from concourse.masks import make_identity

F32 = mybir.dt.float32
F32R = mybir.dt.float32r

B = 4
S = 2048
H = 1024
NH = 16
HD = 64
NSEQ = B * S  # 8192
NCORES = 8
CSLICE = H // NCORES  # 128 hidden cols per core = 2 heads
CHUNK = 512  # seq columns per projection chunk
KCH = H // 128  # 8 contraction tiles for projections
KT = S // 128  # 16 key tiles per (b, h)
QC = S // CHUNK  # 4 query chunks per (b, h)
EXPW = 1024  # exp tile width (2 psum banks)
VW = HD + 1  # V' tile width per key tile

_STATE = None


def _build():
    nc = bacc.Bacc("TRN2", target_bir_lowering=False, debug=False,
                   num_devices=NCORES)

    x = nc.dram_tensor("x", [NSEQ, H], F32, kind="ExternalInput").ap()
    ws = {n: nc.dram_tensor(f"w{n}", [H, CSLICE], F32, kind="ExternalInput").ap()
          for n in "qkv"}
    bs = {n: nc.dram_tensor(f"b{n}", [CSLICE, 1], F32, kind="ExternalInput").ap()
          for n in "qkv"}
    out = nc.dram_tensor("out", [NSEQ, CSLICE], F32, kind="ExternalOutput").ap()

    with tile.TileContext(nc) as tc:
        with (
            tc.tile_pool(name="persist", bufs=1) as persist,
            tc.tile_pool(name="qkvt", bufs=2) as qkvt_pool,
            tc.tile_pool(name="xn", bufs=8) as xn_pool,
            tc.tile_pool(name="xt", bufs=2) as xt_pool,
            tc.tile_pool(name="vp", bufs=4) as vp_pool,
            tc.tile_pool(name="prob", bufs=12) as prob_pool,
            tc.tile_pool(name="cx", bufs=4) as cx_pool,
            tc.tile_pool(name="ot", bufs=6) as ot_pool,
            tc.tile_pool(name="rc", bufs=8) as rc_pool,
            tc.tile_pool(name="smpsum", bufs=2, space="PSUM") as smpsum,
            tc.tile_pool(name="ppsum", bufs=1, space="PSUM") as ppsum,
            tc.tile_pool(name="spsum", bufs=2, space="PSUM") as spsum,
            tc.tile_pool(name="cpsum", bufs=1, space="PSUM") as cpsum,
        ):
            ident = persist.tile([128, 128], F32)
            make_identity(nc, ident)
            # 64x64 identity replicated in both partition halves, so
            # transposes of head-1 slices (base partition 64) have a
            # same-base permutation rhs.
            ident2 = persist.tile([128, HD], F32)
            make_identity(nc, ident2[0:HD, :])
            make_identity(nc, ident2[HD:128, :])
            ones = persist.tile([128, 1], F32)
            nc.vector.memset(ones, 1.0)

            # weights on the ACT DMA queue so they don't block X loads
            wt = {}  # weight k-tiles, lhsT layout [k 128, out 128]
            for n in "qkv":
                for kk in range(KCH):
                    stg = ot_pool.tile([128, CSLICE], F32,
                                       tag="wstage", name="wstage")
                    nc.scalar.dma_start(stg, ws[n][kk * 128:(kk + 1) * 128, :])
                    t = persist.tile([128, CSLICE], F32R,
                                     tag=f"w{n}{kk}", name=f"w{n}{kk}")
                    nc.vector.tensor_copy(t, stg)
                    wt[n, kk] = t
            bt = {}
            for n in "qkv":
                t = persist.tile([128, 1], F32, tag=f"b{n}", name=f"b{n}")
                nc.scalar.dma_start(t, bs[n])
                bt[n] = t

            def alloc_qkvT():
                # per-batch Q^T/K^T/V^T for this core's 2 heads: [128, 2048]
                return {n: qkvt_pool.tile([128, S], F32R if n in "qk" else F32,
                                          tag=f"{n}T", name=f"{n}T")
                        for n in "qkv"}

            def project_chunk_a(qkvT, ci, carry):
                    xns = []
                    for st in range(CHUNK // 128):
                        xn = xn_pool.tile([128, H], F32, tag="xn", name="xn")
                        r0 = ci * CHUNK + st * 128
                        nc.sync.dma_start(xn, x[r0:r0 + 128, :])
                        xns.append(xn)
                    xts = []
                    for kk in range(KCH):
                        tpw = smpsum.tile([128, CHUNK], F32,
                                          tag="sm", name="tpw")
                        for st in range(CHUNK // 128):
                            nc.tensor.transpose(
                                tpw[:, st * 128:(st + 1) * 128],
                                xns[st][:, kk * 128:(kk + 1) * 128], ident)
                        xt = xt_pool.tile([128, CHUNK], F32R,
                                          tag=f"xt{kk}", name=f"xt{kk}")
                        nc.vector.tensor_copy(xt, tpw)
                        xts.append(xt)
                    carry[ci] = xts

            def project_chunk_b(qkvT, ci, carry):
                    j = ci % QC
                    xts = carry.pop(ci)
                    for n in "qkv":
                        ps = ppsum.tile([128, CHUNK], F32,
                                        tag="ps", name=f"ps{n}")
                        for kk in range(KCH):
                            nc.tensor.matmul(
                                ps, wt[n, kk], xts[kk],
                                start=(kk == 0), stop=(kk == KCH - 1))
                        nc.vector.tensor_scalar_add(
                            qkvT[n][:, j * CHUNK:(j + 1) * CHUNK],
                            ps, bt[n])

            def prep_v(qkvT, hl):
                p0 = hl * HD
                vT = qkvT["v"][p0:p0 + HD, :]
                vp = vp_pool.tile([128, KT * VW], F32R, tag="vp", name="vp")
                nc.vector.tensor_copy(
                    vp[:, HD::VW], ones.to_broadcast([128, KT]))
                for kt in range(KT):
                    vtp = smpsum.tile([128, HD], F32, tag="sm", name="vtp")
                    nc.tensor.transpose(
                        vtp, vT[:, kt * 128:(kt + 1) * 128],
                        ident2[p0:p0 + HD, :])
                    nc.vector.tensor_copy(
                        vp[:, kt * VW:kt * VW + HD], vtp)
                return vp

            def attend_qc(qkvT, b, hl, vp, qc):
                    p0 = hl * HD      # partition offset of this head
                    c0 = b * S        # column offset of this batch
                    qT = qkvT["q"][p0:p0 + HD, :]
                    kTt = qkvT["k"][p0:p0 + HD, :]
                    ctx_ps = cpsum.tile([VW, CHUNK], F32,
                                        tag="ctx", name="ctx")
                    rhs_q = qT[:, qc * CHUNK:(qc + 1) * CHUNK]
                    for kp in range(KT // 2):  # pairs of key tiles
                        s_ps = spsum.tile([128, EXPW], F32, tag="s", name="s")
                        with tc.high_priority(offset=150):
                            for half in range(2):
                                kt = kp * 2 + half
                                nc.tensor.matmul(
                                    s_ps[:, half * CHUNK:(half + 1) * CHUNK],
                                    kTt[:, kt * 128:(kt + 1) * 128],
                                    rhs_q, start=True, stop=True)
                        pr = prob_pool.tile([128, EXPW], F32R,
                                            tag="pr", name="pr")
                        nc.scalar.activation(
                            pr, s_ps, mybir.ActivationFunctionType.Exp,
                            scale=1.0 / np.sqrt(float(HD)))
                        for half in range(2):
                            kt = kp * 2 + half
                            nc.tensor.matmul(
                                ctx_ps,
                                vp[:, kt * VW:(kt + 1) * VW],
                                pr[:, half * CHUNK:(half + 1) * CHUNK],
                                start=(kt == 0), stop=(kt == KT - 1))
                    cx = cx_pool.tile([VW, CHUNK], F32, tag="cx", name="cx")
                    with tc.high_priority(offset=150):
                        nc.vector.tensor_copy(cx, ctx_ps)
                    # transpose all 4 q-subtiles into one psum tile, then
                    # one strided reciprocal + 4 normalizing copies
                    otp = smpsum.tile([128, 4 * VW], F32, tag="sm", name="otp")
                    for qt in range(CHUNK // 128):
                        nc.tensor.transpose(
                            otp[:, qt * VW:(qt + 1) * VW],
                            cx[:, qt * 128:(qt + 1) * 128],
                            ident[0:VW, 0:VW])
                    rc = rc_pool.tile([128, 4], F32, tag="rc", name="rc")
                    nc.vector.reciprocal(rc, otp[:, HD::VW])
                    for qt in range(CHUNK // 128):
                        ot = ot_pool.tile([128, HD], F32, tag="ot", name="ot")
                        nc.vector.tensor_scalar_mul(
                            ot, otp[:, qt * VW:qt * VW + HD],
                            rc[:, qt:qt + 1])
                        r0 = c0 + qc * CHUNK + qt * 128
                        nc.sync.dma_start(
                            out[r0:r0 + 128, p0:p0 + HD], ot)

            def att_steps(qkvT, b, hl, vp):
                return [lambda qc=qc: attend_qc(qkvT, b, hl, vp, qc)
                        for qc in range(QC)]

            # software-pipelined emission: projection + V'-prep of batch
            # b+1 are emitted between the ACT-bound attention q-chunks of
            # batch b, giving the list scheduler adjacent independent work
            vps = {}
            qkvTs = {}
            carry = {}
            qkvTs[0] = alloc_qkvT()
            for ci in range(QC):
                project_chunk_a(qkvTs[0], ci, carry)
                project_chunk_b(qkvTs[0], ci, carry)
            vps[0, 0] = prep_v(qkvTs[0], 0)
            vps[0, 1] = prep_v(qkvTs[0], 1)
            for b in range(B):
                att = (att_steps(qkvTs[b], b, 0, vps[b, 0])
                       + att_steps(qkvTs[b], b, 1, vps[b, 1]))
                nxt = []
                if b + 1 < B:
                    qkvTs[b + 1] = alloc_qkvT()
                    for ci in range(QC * (b + 1), QC * (b + 2)):
                        nxt.append(lambda ci=ci: project_chunk_a(
                            qkvTs[b + 1], ci, carry))
                        nxt.append(lambda ci=ci: project_chunk_b(
                            qkvTs[b + 1], ci, carry))
                    nxt.append(lambda: vps.__setitem__(
                        (b + 1, 0), prep_v(qkvTs[b + 1], 0)))
                    nxt.append(lambda: vps.__setitem__(
                        (b + 1, 1), prep_v(qkvTs[b + 1], 1)))
                # 8 att steps, up to 10 nxt steps: round-robin interleave
                order = []
                ai, ni = 0, 0
                while ai < len(att) or ni < len(nxt):
                    if ai < len(att):
                        order.append(att[ai]); ai += 1
                    if ni < len(nxt):
                        order.append(nxt[ni]); ni += 1
                    if ni < len(nxt) and len(nxt) > len(att):
                        order.append(nxt[ni]); ni += 1
                for step in order:
                    step()

    nc.compile()
    return nc


def _get_nc():
    global _STATE
    if _STATE is None:
        _STATE = _build()
    return _STATE


def _in_maps(inputs):
    xf = np.ascontiguousarray(
        np.asarray(inputs["hidden_states"], dtype=np.float32).reshape(NSEQ, H))
    maps = []
    for c in range(NCORES):
        sl = slice(c * CSLICE, (c + 1) * CSLICE)
        m = {"x": xf}
        for n, wkey, bkey in (("q", "Wq", "bq"), ("k", "Wk", "bk"),
                              ("v", "Wv", "bv")):
            m[f"w{n}"] = np.ascontiguousarray(
                np.asarray(inputs[wkey], dtype=np.float32)[:, sl])
            m[f"b{n}"] = np.ascontiguousarray(
                np.asarray(inputs[bkey], dtype=np.float32)[sl].reshape(
                    CSLICE, 1))
        maps.append(m)
    return maps


def _assemble(results):
    parts = [results[c]["out"].reshape(B, S, CSLICE) for c in range(NCORES)]
    return np.ascontiguousarray(np.concatenate(parts, axis=-1))


def _run(inputs, trace=False):
    nc = _get_nc()
    res = run_bass_kernel_spmd(nc, _in_maps(inputs),
                               core_ids=list(range(NCORES)), trace=trace)
    return _assemble(res.results), res


def kernel(**inputs):
    out, _ = _run(inputs, trace=False)
    return out


def run_traced(**inputs):
    out, res = _run(inputs, trace=True)
    return out, res


# revision 53
# speedup vs baseline: 1.0051x; 1.0051x over previous
"""BERT self-attention (no mask) on 8 TRN2 NeuronCores, head-parallel.

Full inputs in, full output out. Core c computes heads 2c and 2c+1, i.e.
output hidden columns [c*128, (c+1)*128). Matmul operands are float32r
(full-rate near-fp32 streaming; producers round on write). Attention is
computed in transposed layout (scores^T[k, q]) so the softmax
denominator comes out of the PV matmul for free via a ones-column
appended to V. Projection (per batch) and attention (previous batch)
are interleaved so TensorE fills the gaps of the ACT-bound exp stream.
"""

import numpy as np

try:
    import concourse.bass as bass
except ImportError:  # toolchain not on sys.path in the caller's environment
    import sys
    sys.path.insert(0, "/opt/trn_rl_repo")
    import concourse.bass as bass
import concourse.bacc as bacc
import concourse.mybir as mybir
import concourse.tile as tile
from concourse.bass_utils import run_bass_kernel_spmd
from concourse.masks import make_identity

F32 = mybir.dt.float32
F32R = mybir.dt.float32r

B = 4
S = 2048
H = 1024
NH = 16
HD = 64
NSEQ = B * S  # 8192
NCORES = 8
CSLICE = H // NCORES  # 128 hidden cols per core = 2 heads
CHUNK = 512  # seq columns per projection chunk
KCH = H // 128  # 8 contraction tiles for projections
KT = S // 128  # 16 key tiles per (b, h)
QC = S // CHUNK  # 4 query chunks per (b, h)
EXPW = 1024  # exp tile width (2 psum banks)
VW = HD + 1  # V' tile width per key tile

_STATE = None


def _build():
    nc = bacc.Bacc("TRN2", target_bir_lowering=False, debug=False,
                   num_devices=NCORES)

    x = nc.dram_tensor("x", [NSEQ, H], F32, kind="ExternalInput").ap()
    ws = {n: nc.dram_tensor(f"w{n}", [H, CSLICE], F32, kind="ExternalInput").ap()
          for n in "qkv"}
    bs = {n: nc.dram_tensor(f"b{n}", [CSLICE, 1], F32, kind="ExternalInput").ap()
          for n in "qkv"}
    out = nc.dram_tensor("out", [NSEQ, CSLICE], F32, kind="ExternalOutput").ap()

    with tile.TileContext(nc) as tc:
        with (
            tc.tile_pool(name="persist", bufs=1) as persist,
            tc.tile_pool(name="qkvt", bufs=2) as qkvt_pool,
            tc.tile_pool(name="xn", bufs=8) as xn_pool,
            tc.tile_pool(name="xt", bufs=2) as xt_pool,
            tc.tile_pool(name="vp", bufs=4) as vp_pool,
            tc.tile_pool(name="prob", bufs=12) as prob_pool,
            tc.tile_pool(name="cx", bufs=4) as cx_pool,
            tc.tile_pool(name="ot", bufs=6) as ot_pool,
            tc.tile_pool(name="rc", bufs=8) as rc_pool,
            tc.tile_pool(name="smpsum", bufs=2, space="PSUM") as smpsum,
            tc.tile_pool(name="ppsum", bufs=1, space="PSUM") as ppsum,
            tc.tile_pool(name="spsum", bufs=2, space="PSUM") as spsum,
            tc.tile_pool(name="cpsum", bufs=1, space="PSUM") as cpsum,
        ):
            ident = persist.tile([128, 128], F32)
            make_identity(nc, ident)
            # 64x64 identity replicated in both partition halves, so
            # transposes of head-1 slices (base partition 64) have a
            # same-base permutation rhs.
            ident2 = persist.tile([128, HD], F32)
            make_identity(nc, ident2[0:HD, :])
            make_identity(nc, ident2[HD:128, :])
            ones = persist.tile([128, 1], F32)
            nc.vector.memset(ones, 1.0)

            # weights on the ACT DMA queue so they don't block X loads
            wt = {}  # weight k-tiles, lhsT layout [k 128, out 128]
            for n in "qkv":
                for kk in range(KCH):
                    stg = ot_pool.tile([128, CSLICE], F32,
                                       tag="wstage", name="wstage")
                    nc.scalar.dma_start(stg, ws[n][kk * 128:(kk + 1) * 128, :])
                    t = persist.tile([128, CSLICE], F32R,
                                     tag=f"w{n}{kk}", name=f"w{n}{kk}")
                    nc.vector.tensor_copy(t, stg)
                    wt[n, kk] = t
            bt = {}
            for n in "qkv":
                t = persist.tile([128, 1], F32, tag=f"b{n}", name=f"b{n}")
                nc.scalar.dma_start(t, bs[n])
                bt[n] = t

            def alloc_qkvT():
                # per-batch Q^T/K^T/V^T for this core's 2 heads: [128, 2048]
                return {n: qkvt_pool.tile([128, S], F32R if n in "qk" else F32,
                                          tag=f"{n}T", name=f"{n}T")
                        for n in "qkv"}

            def project_chunk_a(qkvT, ci, carry):
                    xns = []
                    for st in range(CHUNK // 128):
                        xn = xn_pool.tile([128, H], F32, tag="xn", name="xn")
                        r0 = ci * CHUNK + st * 128
                        nc.sync.dma_start(xn, x[r0:r0 + 128, :])
                        xns.append(xn)
                    xts = []
                    for kk in range(KCH):
                        tpw = smpsum.tile([128, CHUNK], F32,
                                          tag="sm", name="tpw")
                        for st in range(CHUNK // 128):
                            nc.tensor.transpose(
                                tpw[:, st * 128:(st + 1) * 128],
                                xns[st][:, kk * 128:(kk + 1) * 128], ident)
                        xt = xt_pool.tile([128, CHUNK], F32R,
                                          tag=f"xt{kk}", name=f"xt{kk}")
                        nc.vector.tensor_copy(xt, tpw)
                        xts.append(xt)
                    carry[ci] = xts

            def project_chunk_b(qkvT, ci, carry):
                    j = ci % QC
                    xts = carry.pop(ci)
                    for n in "qkv":
                        ps = ppsum.tile([128, CHUNK], F32,
                                        tag="ps", name=f"ps{n}")
                        for kk in range(KCH):
                            nc.tensor.matmul(
                                ps, wt[n, kk], xts[kk],
                                start=(kk == 0), stop=(kk == KCH - 1))
                        nc.vector.tensor_scalar_add(
                            qkvT[n][:, j * CHUNK:(j + 1) * CHUNK],
                            ps, bt[n])

            def prep_v(qkvT, hl):
                p0 = hl * HD
                vT = qkvT["v"][p0:p0 + HD, :]
                vp = vp_pool.tile([128, KT * VW], F32R, tag="vp", name="vp")
                nc.vector.tensor_copy(
                    vp[:, HD::VW], ones.to_broadcast([128, KT]))
                for kt in range(KT):
                    vtp = smpsum.tile([128, HD], F32, tag="sm", name="vtp")
                    nc.tensor.transpose(
                        vtp, vT[:, kt * 128:(kt + 1) * 128],
                        ident2[p0:p0 + HD, :])
                    nc.vector.tensor_copy(
                        vp[:, kt * VW:kt * VW + HD], vtp)
                return vp

            def attend_qc(qkvT, b, hl, vp, qc):
                    p0 = hl * HD      # partition offset of this head
                    c0 = b * S        # column offset of this batch
                    qT = qkvT["q"][p0:p0 + HD, :]
                    kTt = qkvT["k"][p0:p0 + HD, :]
                    ctx_ps = cpsum.tile([VW, CHUNK], F32,
                                        tag="ctx", name="ctx")
                    rhs_q = qT[:, qc * CHUNK:(qc + 1) * CHUNK]
                    for kp in range(KT // 2):  # pairs of key tiles
                        s_ps = spsum.tile([128, EXPW], F32, tag="s", name="s")
                        with tc.high_priority(offset=150):
                            for half in range(2):
                                kt = kp * 2 + half
                                nc.tensor.matmul(
                                    s_ps[:, half * CHUNK:(half + 1) * CHUNK],
                                    kTt[:, kt * 128:(kt + 1) * 128],
                                    rhs_q, start=True, stop=True)
                        pr = prob_pool.tile([128, EXPW], F32R,
                                            tag="pr", name="pr")
                        nc.scalar.activation(
                            pr, s_ps, mybir.ActivationFunctionType.Exp,
                            scale=1.0 / np.sqrt(float(HD)))
                        for half in range(2):
                            kt = kp * 2 + half
                            nc.tensor.matmul(
                                ctx_ps,
                                vp[:, kt * VW:(kt + 1) * VW],
                                pr[:, half * CHUNK:(half + 1) * CHUNK],
                                start=(kt == 0), stop=(kt == KT - 1))
                    cx = cx_pool.tile([VW, CHUNK], F32, tag="cx", name="cx")
                    with tc.high_priority(offset=150):
                        nc.vector.tensor_copy(cx, ctx_ps)
                    # transpose all 4 q-subtiles into one psum tile, then
                    # one strided reciprocal + 4 normalizing copies
                    otp = smpsum.tile([128, 4 * VW], F32, tag="sm", name="otp")
                    for qt in range(CHUNK // 128):
                        nc.tensor.transpose(
                            otp[:, qt * VW:(qt + 1) * VW],
                            cx[:, qt * 128:(qt + 1) * 128],
                            ident[0:VW, 0:VW])
                    rc = rc_pool.tile([128, 4], F32, tag="rc", name="rc")
                    nc.vector.reciprocal(rc, otp[:, HD::VW])
                    for qt in range(CHUNK // 128):
                        ot = ot_pool.tile([128, HD], F32, tag="ot", name="ot")
                        nc.vector.tensor_scalar_mul(
                            ot, otp[:, qt * VW:qt * VW + HD],
                            rc[:, qt:qt + 1])
                        r0 = c0 + qc * CHUNK + qt * 128
                        nc.sync.dma_start(
                            out[r0:r0 + 128, p0:p0 + HD], ot)

            def att_steps(qkvT, b, hl, vp):
                return [lambda qc=qc: attend_qc(qkvT, b, hl, vp, qc)
                        for qc in range(QC)]

            # software-pipelined emission: projection + V'-prep of batch
            # b+1 are emitted between the ACT-bound attention q-chunks of
            # batch b, giving the list scheduler adjacent independent work
            vps = {}
            qkvTs = {}
            carry = {}
            qkvTs[0] = alloc_qkvT()
            for ci in range(QC):
                project_chunk_a(qkvTs[0], ci, carry)
                project_chunk_b(qkvTs[0], ci, carry)
            vps[0, 0] = prep_v(qkvTs[0], 0)
            vps[0, 1] = prep_v(qkvTs[0], 1)
            for b in range(B):
                att = (att_steps(qkvTs[b], b, 0, vps[b, 0])
                       + att_steps(qkvTs[b], b, 1, vps[b, 1]))
                nxt = []
                if b + 1 < B:
                    qkvTs[b + 1] = alloc_qkvT()
                    for ci in range(QC * (b + 1), QC * (b + 2)):
                        nxt.append(lambda ci=ci: project_chunk_a(
                            qkvTs[b + 1], ci, carry))
                        nxt.append(lambda ci=ci: project_chunk_b(
                            qkvTs[b + 1], ci, carry))
                    nxt.append(lambda: vps.__setitem__(
                        (b + 1, 0), prep_v(qkvTs[b + 1], 0)))
                    nxt.append(lambda: vps.__setitem__(
                        (b + 1, 1), prep_v(qkvTs[b + 1], 1)))
                # 8 att steps, up to 10 nxt steps: round-robin interleave
                order = list(att[:2])
                ai, ni = 2, 0
                while ai < len(att) or ni < len(nxt):
                    if ai < len(att):
                        order.append(att[ai]); ai += 1
                    for _ in range(2):
                        if ni < len(nxt):
                            order.append(nxt[ni]); ni += 1
                for step in order:
                    step()

    nc.compile()
    return nc


def _get_nc():
    global _STATE
    if _STATE is None:
        _STATE = _build()
    return _STATE


def _in_maps(inputs):
    xf = np.ascontiguousarray(
        np.asarray(inputs["hidden_states"], dtype=np.float32).reshape(NSEQ, H))
    maps = []
    for c in range(NCORES):
        sl = slice(c * CSLICE, (c + 1) * CSLICE)
        m = {"x": xf}
        for n, wkey, bkey in (("q", "Wq", "bq"), ("k", "Wk", "bk"),
                              ("v", "Wv", "bv")):
            m[f"w{n}"] = np.ascontiguousarray(
                np.asarray(inputs[wkey], dtype=np.float32)[:, sl])
            m[f"b{n}"] = np.ascontiguousarray(
                np.asarray(inputs[bkey], dtype=np.float32)[sl].reshape(
                    CSLICE, 1))
        maps.append(m)
    return maps


def _assemble(results):
    parts = [results[c]["out"].reshape(B, S, CSLICE) for c in range(NCORES)]
    return np.ascontiguousarray(np.concatenate(parts, axis=-1))


def _run(inputs, trace=False):
    nc = _get_nc()
    res = run_bass_kernel_spmd(nc, _in_maps(inputs),
                               core_ids=list(range(NCORES)), trace=trace)
    return _assemble(res.results), res


def kernel(**inputs):
    out, _ = _run(inputs, trace=False)
    return out


def run_traced(**inputs):
    out, res = _run(inputs, trace=True)
    return out, res
